# revision 75
# baseline (speedup 1.0000x reference)
"""Trainium2 Bass kernel for nn_Decode_Layer (dense transformer decode layer).

Strategy (8 NeuronCores, SPMD):
  - Sequence-parallel: core c owns position chunks {c, 15-c} (128 pos each) of
    both batches -> 512 tokens/core.  Balanced causal work.
  - Residual stream token-major (tm) in SBUF; normed activations transposed to
    feature-major (fm) via PE transposes for the matmuls.
  - K/V for both attention blocks computed on own shard, AllGather'd (4 small
    collectives) in shard-major token order; attention consumes shard-major
    order directly (order-independent reductions; causality via host-built
    per-core additive masks so the instruction stream is core-uniform).
  - Softmax without max-subtraction (scores provably small here); a ones
    column interleaved into V makes the softmax denominator fall out of the
    same AV matmul (row 64 of the psum).
  - Q/K/V projections in fp8e4m3 + DoubleRow perf mode (weights host-prescaled
    by 32, compensated in the psum-evacuation scales).  Wo and the whole FFN
    stay bf16: fp8 anywhere on the FFN path (or more than Q/K/V) pushes the
    absmax error past the 2e-2 gate (measured by quantization simulation).
  - Causal masks are multiplicative 0/1 (exp(s+m) == exp(s)*m01), applied
    post-exp on DVE; cross-attn position bias via precomputed exp(pos) and a
    DVE bf16 2x multiply -- both remove bias matmuls from the PE stream.
  - Exps merged into [128,1024] two-bank psum tiles (one ACT op per pair).
  - rms h2/h3 transposed on the DMA xbar (dma_start_transpose) instead of PE.
  - V gathered per (head-quad, batch) with 1040B-contiguous DMA rows; first
    K/V gathers of each attention block hoisted to overlap prior phases.
"""
import numpy as np

B, L, D, H, HD, FFN = 2, 2048, 1024, 16, 64, 4096
P = 128
NC = 8
DT = D // P            # 8 feature tiles
KF = FFN // P          # 32 ffn tiles
EPS = 1e-6
WS = 32.0              # fp8 weight prescale
BLOCKS = [(b, q2) for b in range(B) for q2 in range(2)]  # own token block order

_CACHE = {}


def _build(timeline=False, stages=99):
    import concourse.bacc as bacc
    import concourse.mybir as mybir
    import concourse.tile as tile
    from concourse import masks
    from contextlib import ExitStack

    f32 = mybir.dt.float32
    f32r = mybir.dt.float32r
    bf16 = mybir.dt.bfloat16
    f8 = mybir.dt.float8e4
    AF = mybir.ActivationFunctionType
    ALU = mybir.AluOpType
    DR = mybir.MatmulPerfMode.DoubleRow

    nc = bacc.Bacc("TRN2", target_bir_lowering=False, debug=False,
                   num_devices=(1 if timeline else NC))

    # ---------------- I/O ----------------
    x_own = nc.dram_tensor("x_own", [4, P, D], f32, kind="ExternalInput")
    mem_own = nc.dram_tensor("mem_own", [4, P, D], bf16, kind="ExternalInput")
    pos_own = nc.dram_tensor("pos_own", [2, P, L], bf16, kind="ExternalInput")
    smask = nc.dram_tensor("smask", [2, NC, P, 256], bf16, kind="ExternalInput")
    # fp8 prepacked projection weights (see _pack_fm/_pack_tm)
    WQ8 = [nc.dram_tensor(n, [P, DT * 1024], f8, kind="ExternalInput")
           for n in ("Wq8_1", "Wq8_2")]
    WK8 = [nc.dram_tensor(n, [P, DT * 1024], f8, kind="ExternalInput")
           for n in ("Wk8_1", "Wk8_2")]
    WV8 = [nc.dram_tensor(n, [P, 2 * 4096], f8, kind="ExternalInput")
           for n in ("Wv8_1", "Wv8_2")]
    WOb = [nc.dram_tensor(n, [D, D], bf16, kind="ExternalInput")
           for n in ("Wo1b", "Wo2b")]
    W1b = nc.dram_tensor("W1b", [P, KF * DT * P], bf16, kind="ExternalInput")
    W2b = nc.dram_tensor("W2b", [FFN, D], bf16, kind="ExternalInput")
    out = nc.dram_tensor("out", [4, P, D], f32, kind="ExternalOutput")

    with tile.TileContext(nc) as tc, ExitStack() as g:
        # ---- global pools
        single = g.enter_context(tc.tile_pool(name="single", bufs=1))
        resid = g.enter_context(tc.tile_pool(name="resid", bufs=2))
        wlhsp = g.enter_context(tc.tile_pool(name="wlhsp", bufs=2))
        wrhsp = g.enter_context(tc.tile_pool(name="wrhsp", bufs=2))
        psp = g.enter_context(tc.tile_pool(name="psp", bufs=7, space="PSUM"))
        evacp = g.enter_context(tc.tile_pool(name="evacp", bufs=3))
        sqp = g.enter_context(tc.tile_pool(name="sqp", bufs=2))
        tmqb = g.enter_context(tc.tile_pool(name="tmqb", bufs=2))
        smallp = g.enter_context(tc.tile_pool(name="smallp", bufs=12))
        normp = g.enter_context(tc.tile_pool(name="normp", bufs=8))
        dram = g.enter_context(tc.tile_pool(name="dram", bufs=1, space="DRAM"))

        # collective buffers (k: [feat, tok]; v: [tok, head*65 interleaved+ones])
        cc_k_in = [dram.tile([D, 512], bf16, name=f"cck{i}") for i in range(2)]
        cc_v_in = [dram.tile([512, H * 65], bf16, name=f"ccv{i}") for i in range(2)]
        cc_k_out = [dram.tile([NC * D, 512], bf16, addr_space="Shared",
                              name=f"ccko{i}") for i in range(2)]
        cc_v_out = [dram.tile([NC * 512, H * 65], bf16, addr_space="Shared",
                              name=f"ccvo{i}") for i in range(2)]

        # ---- constants / small loads
        ident = single.tile([P, P], f32)
        masks.make_identity(nc, ident[:])
        ident_b = single.tile([P, P], bf16)
        nc.vector.tensor_copy(ident_b[:], ident[:])
        ones1_b = single.tile([1, HD], bf16)
        nc.gpsimd.memset(ones1_b[:], 1.0)
        smk = single.tile([P, 2, NC, 256], bf16)
        nc.gpsimd.dma_start(smk[:], smask.ap().rearrange("t s p q -> p t s q"))
        posE = single.tile([P, 16, 256], bf16)

        # ---- helpers
        # PSUM budget (16KB/partition): pb 2x2KB + pbw 2x4KB + po 2KB + psb 2KB
        def mmslot(shape=(P, 512)):
            return psp.tile(list(shape), f32, tag="pb", name="ps", bufs=2)

        def mmwide():
            return psp.tile([P, 1024], f32, tag="pbw", name="psw", bufs=2)

        def mmout():
            return psp.tile([65, 256], f32, tag="po", name="pso", bufs=2)

        def rms_to_fm(r_tm, h_out, dma_t=False):
            """r_tm [128,4,1024] f32 -> h_out [128,8,512] (fp8 or bf16).
            Per-qb chains so qb0's transposes start before qb3's residual is
            even written.  dma_t: transpose on the DMA xbar instead of PE."""
            for qb in range(4):
                sq = sqp.tile([P, D], f32, tag="sq", name="sq")
                ms1 = smallp.tile([P, 1], f32, tag="ms", name="ms1")
                nc.scalar.activation(sq[:], r_tm[:, qb, :], AF.Square,
                                     accum_out=ms1[:])
                msn = smallp.tile([P, 1], f32, tag="ms", name="msn")
                nc.vector.tensor_scalar(msn[:], ms1[:], 1.0 / D, EPS,
                                        op0=ALU.mult, op1=ALU.add)
                msq = smallp.tile([P, 1], f32, tag="ms", name="msq")
                nc.scalar.activation(msq[:], msn[:], AF.Sqrt)
                rinv = smallp.tile([P, 1], f32, tag="ms", name="rinv")
                nc.vector.reciprocal(rinv[:], msq[:])
                if dma_t:
                    ht = tmqb.tile([P, D], bf16, tag="tmqbb", name="htb",
                                   bufs=2)
                    nc.vector.tensor_scalar_mul(ht[:], r_tm[:, qb, :], rinv[:])
                    if h_out.dtype == bf16:
                        nc.sync.dma_start_transpose(
                            h_out[:, :, qb * P:(qb + 1) * P], ht[:])
                    else:
                        hTb = tmqb.tile([P, DT, P], bf16, tag="hTb", name="hTb",
                                        bufs=2)
                        nc.sync.dma_start_transpose(hTb[:], ht[:])
                        nc.vector.tensor_copy(h_out[:, :, qb * P:(qb + 1) * P],
                                              hTb[:])
                else:
                    ht = tmqb.tile([P, D], f32, tag="tmqb", name="ht")
                    nc.vector.tensor_scalar_mul(ht[:], r_tm[:, qb, :], rinv[:])
                    for d in range(DT):
                        pt = mmslot((P, P))
                        nc.tensor.transpose(pt[:], ht[:, d * P:(d + 1) * P],
                                            ident[:])
                        nc.vector.tensor_copy(h_out[:, d, qb * P:(qb + 1) * P],
                                              pt[:])

        def proj_fm_dr(w8_dram, src8, sink, T=DT, tok_split=False):
            """psum[t] [128 dout, 512 tok] = W.T @ src via fp8 DoubleRow.
            tok_split: emit token-half groups so the first half's matmuls can
            run before the later qb's of src are even produced."""
            wv = w8_dram.ap().rearrange("p (t i j m) -> p t i j m",
                                        t=T, i=4, j=2)
            wt = wlhsp.tile([P, T, 4, 2, P], f8, tag="wlhs", name="wt")
            nc.sync.dma_start(wt[:], wv)
            for t in range(T):
                ps = mmslot()
                if tok_split:
                    for hf in range(2):
                        for i in range(4):
                            nc.tensor.matmul(
                                ps[:, 256 * hf:256 * hf + 256], wt[:, t, i],
                                src8[:, 2 * i:2 * i + 2,
                                     256 * hf:256 * hf + 256],
                                start=(i == 0), stop=(i == 3),
                                perf_mode=DR, skip_group_check=True)
                else:
                    for i in range(4):
                        nc.tensor.matmul(ps[:], wt[:, t, i],
                                         src8[:, 2 * i:2 * i + 2, :],
                                         start=(i == 0), stop=(i == 3),
                                         perf_mode=DR)
                sink(t, ps)

        def proj_tm_dr(w8_dram, src8, sink):
            """psum[qb,fh] [128 tok, 512 feat] = src.T @ W via fp8 DoubleRow."""
            wv = w8_dram.ap().rearrange("p (f i j n) -> p f i j n", f=2, i=4, j=2)
            for fh in range(2):
                wr = wrhsp.tile([P, 4, 2, 512], f8, tag="wrhs", name="wr")
                nc.sync.dma_start(wr[:], wv[:, fh])
                for qp in range(2):           # qb pairs to cap live psums at 2
                    pss = [mmslot() for _ in range(2)]
                    for i in range(4):
                        for qi in range(2):
                            qb = 2 * qp + qi
                            nc.tensor.matmul(
                                pss[qi][:],
                                src8[:, 2 * i:2 * i + 2, qb * P:(qb + 1) * P],
                                wr[:, i], start=(i == 0), stop=(i == 3),
                                perf_mode=DR)
                    for qi in range(2):
                        sink(2 * qp + qi, fh, pss[qi])

        def proj_tm_bf(w_dram, src_fm, sink):
            """bf16 token-major projection (used for Wo): psum[qb,fh] =
            src_fm.T @ W.  Preloads the fh-half of W, then two qb-pair
            passes reusing it (keeps live psums at 2)."""
            for fh in range(2):
                wrs = []
                for dk in range(DT):
                    wr = wrhsp.tile([P, 512], bf16, tag="wrhsb", name="wrb",
                                    bufs=9)
                    nc.sync.dma_start(
                        wr[:], w_dram[dk * P:(dk + 1) * P,
                                      fh * 512:(fh + 1) * 512])
                    wrs.append(wr)
                for qp in range(2):
                    pss = [mmslot() for _ in range(2)]
                    for dk in range(DT):
                        for qi in range(2):
                            qb = 2 * qp + qi
                            nc.tensor.matmul(pss[qi][:],
                                             src_fm[:, dk, qb * P:(qb + 1) * P],
                                             wrs[dk][:], start=(dk == 0),
                                             stop=(dk == DT - 1))
                    for qi in range(2):
                        sink(2 * qp + qi, fh, pss[qi])

        def attn_load_vq(blk, vhp, dt, b):
            vview = cc_v_out[blk][:].rearrange("(s k p) f -> p s k f", s=NC, p=P)
            vq = vhp.tile([P, NC, 2, 8 * 65], bf16, tag=f"vq{b}", name="vq")
            q0 = (dt // 4) * 8 * 65
            for kq in range(2):
                nc.sync.dma_start(vq[:, :, kq, :],
                                  vview[:, :, 2 * b + kq, q0:q0 + 8 * 65])
            return vq

        def attn_load_kh(blk, khp, dt):
            kview = cc_k_out[blk][:].rearrange("(s d p) t -> d p s t", s=NC, p=P)
            kh = khp.tile([P, NC, 512], bf16, tag="kh", name="kh")
            nc.sync.dma_start(kh[:], kview[dt])
            return kh

        def attention(blk, q_fm, o_fm, bias_self, pools=None, warm=None):
            """blk 0: self (bias_self=True -> smk), 1: cross (pos_T bias)."""
            with ExitStack() as actx:
                if pools is None:
                    khp = actx.enter_context(
                        tc.tile_pool(name=f"khp{blk}", bufs=2))
                    vhp = actx.enter_context(
                        tc.tile_pool(name=f"vhp{blk}", bufs=1))
                else:
                    khp, vhp = pools
                ep = actx.enter_context(tc.tile_pool(name=f"ep{blk}", bufs=6))
                vhq = [None, None]
                if warm is not None:
                    kh0, vhq = warm
                for dt in range(DT):          # head pair dt -> heads 2dt, 2dt+1
                    if dt % 4 == 0 and (dt > 0 or warm is None):
                        for b in range(B):
                            vhq[b] = attn_load_vq(blk, vhp, dt, b)
                    if dt == 0 and warm is not None:
                        kh = kh0
                    else:
                        kh = attn_load_kh(blk, khp, dt)
                    for hi in range(2):
                        h = 2 * dt + hi
                        ho = (h % 8) * 65      # head offset inside vq tile
                        hs = slice(HD * hi, HD * hi + HD)
                        for b in range(B):
                            vh = vhq[b]
                            qa = q_fm[hs, dt, 256 * b:256 * b + 256]
                            if bias_self:
                                self_attn_bh(kh, hs, vh, ho, qa,
                                             q_fm[hs, dt, 128 * (2 * b + 1):
                                                  128 * (2 * b + 1) + 128],
                                             b, h, o_fm, ep)
                            else:
                                cross_attn_bh(kh, hs, vh, ho, qa, b, h, o_fm, ep)

        def self_attn_bh(kh, hs, vh, ho, qa, qb_, b, h, o_fm, ep):
            psOa = mmout()
            first_av = [True]

            def av(lhsT, rhs, cols, last=False):
                nc.tensor.matmul(psOa[:, cols], lhsT, rhs,
                                 start=first_av[0], stop=last,
                                 skip_group_check=True)
                first_av[0] = False

            # kq2 = 0 tiles (k chunk j = s): both q-blocks; 2 wide psums of
            # 4 s-chunks x 256q each, single exp per wide psum.  Causal
            # masking is multiplicative 0/1 masks applied post-exp on DVE
            # (masks only ever apply to the left (chunk-c) halves).
            for w in range(2):
                psA = mmwide()
                ea = ep.tile([P, 4, 256], bf16, tag="ea", name="ea")
                for u in range(2):
                    i = 2 * w + u
                    for t in range(2):
                        sidx = 2 * i + t
                        nc.tensor.matmul(
                            psA[:, 512 * u + 256 * t:512 * u + 256 * t + 256],
                            kh[hs, sidx, 256 * b:256 * b + 128], qa,
                            start=(t == 0), stop=(t == 1), skip_group_check=True)
                nc.scalar.activation(ea[:].rearrange("p a b -> p (a b)"),
                                     psA[:], AF.Exp)
                nc.vector.tensor_mul(ea[:, :, 0:128],
                                     ea[:, :, 0:128],
                                     smk[:, 0, 4 * w:4 * w + 4, 0:128])
                for t in range(4):
                    sidx = 4 * w + t
                    av(vh[:, sidx, 0, ho:ho + 65], ea[:, t, :], slice(0, 256))
            # kq2 = 1 tiles (k chunk j = 15-s): right q-block only; one wide
            # psum of 8 s-chunks x 128q
            psB = mmwide()
            eb = ep.tile([P, 8, P], bf16, tag="ea", name="eb")
            for u in range(2):
                for t in range(4):
                    sidx = 4 * u + t
                    nc.tensor.matmul(
                        psB[:, 512 * u + 128 * t:512 * u + 128 * t + 128],
                        kh[hs, sidx, 256 * b + 128:256 * b + 256],
                        qb_, start=(t == 0), stop=(t == 3), skip_group_check=True)
            nc.scalar.activation(eb[:].rearrange("p a b -> p (a b)"),
                                 psB[:], AF.Exp)
            nc.vector.tensor_mul(eb[:], eb[:], smk[:, 1, :, 0:128])
            for t in range(8):
                av(vh[:, t, 1, ho:ho + 65], eb[:, t, :], slice(128, 256),
                   last=(t == NC - 1))
            finish_attn(psOa, b, h, o_fm)

        def cross_attn_bh(kh, hs, vh, ho, qa, b, h, o_fm, ep):
            psO = mmout()
            for sp in range(4):               # sidx pairs, wide psum each
                psC = mmwide()
                ecr = ep.tile([P, 4, 256], bf16, tag="ea", name="ecr")
                ec = ep.tile([P, 4, 256], bf16, tag="ea", name="ec")
                for u in range(2):
                    sidx = 2 * sp + u
                    for kq2 in range(2):
                        nc.tensor.matmul(
                            psC[:, 512 * u + 256 * kq2:512 * u + 256 * kq2 + 256],
                            kh[hs, sidx,
                               256 * b + 128 * kq2:256 * b + 128 * kq2 + 128],
                            qa, start=(kq2 == 0), stop=(kq2 == 1),
                            skip_group_check=True)
                nc.scalar.activation(ecr[:].rearrange("p a b -> p (a b)"),
                                     psC[:], AF.Exp)
                # bias add via exp(s+m) = exp(s)*exp(m) on DVE (bf16 2x mode)
                nc.vector.tensor_mul(
                    ec[:].rearrange("p a b -> p (a b)"),
                    ecr[:].rearrange("p a b -> p (a b)"),
                    posE[:, 4 * sp:4 * sp + 4, :].rearrange("p a b -> p (a b)"))
                for u in range(2):
                    sidx = 2 * sp + u
                    for kq2 in range(2):
                        nc.tensor.matmul(
                            psO[:], vh[:, sidx, kq2, ho:ho + 65],
                            ec[:, 2 * u + kq2, :],
                            start=(sp == 0 and u == 0 and kq2 == 0),
                            stop=(sp == 3 and u == 1 and kq2 == 1))
            finish_attn(psO, b, h, o_fm)

        def finish_attn(psO, b, h, o_fm):
            rec = normp.tile([1, 256], bf16, tag="rec", name="rec")
            with nc.allow_low_precision(reason="softmax denom recip"):
                nc.vector.reciprocal(rec[:], psO[64:65, :])
            lb = normp.tile([HD, 256], bf16, tag="lb", name="lb")
            nc.gpsimd.partition_broadcast(lb[:], rec[:])
            nc.vector.tensor_mul(
                o_fm[HD * (h % 2):HD * (h % 2) + HD, h // 2, 256 * b:256 * b + 256],
                psO[0:64, :], lb[:])

        def kv_shard(blk, src8):
            """k/v shard projections for block blk from src8 + cc_in DMAs."""
            kview = cc_k_in[blk][:].rearrange("(d p) t -> d p t", p=P)
            vview = cc_v_in[blk][:].rearrange("(q p) f -> q p f", p=P)

            def k_sink(dt, ps):
                ev = evacp.tile([P, 512], bf16, tag="ev", name="kev")
                nc.vector.tensor_scalar_mul(ev[:], ps[:], 1.0 / WS)
                nc.sync.dma_start(kview[dt], ev[:])

            proj_fm_dr(WK8[blk], src8, k_sink)

            vown = [None] * 4
            for qb in range(4):
                vown[qb] = sqp.tile([P, H, 65], bf16, tag="vown", bufs=4, name="vown")
                nc.gpsimd.memset(vown[qb][:, :, 64:65], 1.0)

            def v_sink(qb, fh, ps):
                nc.vector.tensor_scalar_mul(
                    vown[qb][:, fh * 8:(fh + 1) * 8, 0:64],
                    ps[:].rearrange("p (a b) -> p a b", a=8), 1.0 / WS)
                if fh == 1:
                    nc.sync.dma_start(vview[qb],
                                      vown[qb][:].rearrange("p a b -> p (a b)"))

            proj_tm_dr(WV8[blk], src8, v_sink)

        # ================= phase 0 =================
        x_tm = resid.tile([P, 4, D], f32, tag="resid", name="x_tm")

        with ExitStack() as p0:
            q1p = p0.enter_context(tc.tile_pool(name="q1p", bufs=1))
            o1p = p0.enter_context(tc.tile_pool(name="o1p", bufs=1))
            q1_fm = q1p.tile([P, DT, 512], bf16)
            o1_fm = o1p.tile([P, DT, 512], bf16)

            rg = [list(range(NC))]

            def emit_ag(blk):
                if timeline:
                    nc.sync.dma_start(cc_k_out[blk][0:D, :], cc_k_in[blk][:])
                    nc.sync.dma_start(cc_v_out[blk][0:512, :], cc_v_in[blk][:])
                else:
                    nc.gpsimd.collective_compute(
                        "AllGather", ALU.bypass, replica_groups=rg,
                        ins=[cc_k_in[blk][:].opt()], outs=[cc_k_out[blk][:].opt()])
                    nc.gpsimd.collective_compute(
                        "AllGather", ALU.bypass, replica_groups=rg,
                        ins=[cc_v_in[blk][:].opt()], outs=[cc_v_out[blk][:].opt()])

            # block-2 inputs: mem/pos transposes (emitted before rms so PE has
            # work while x/mem DMAs land), projections + AG2 after AG1
            hA2 = p0.enter_context(tc.tile_pool(name="hA2", bufs=1))
            mem8 = hA2.tile([P, DT, 512], f8)
            khp0 = p0.enter_context(tc.tile_pool(name="khp0", bufs=2))
            vhp0 = p0.enter_context(tc.tile_pool(name="vhp0", bufs=1))

            def mem_transposes():
                for qb in range(4):
                    mt = tmqb.tile([P, D], bf16, tag="tmqbb", name="mt", bufs=2)
                    nc.sync.dma_start(mt[:], mem_own[qb, :, :])
                    for d in range(DT):
                        pt = psp.tile([P, P], bf16, tag="pb", name="ptb",
                                      bufs=2)
                        nc.tensor.transpose(pt[:], mt[:, d * P:(d + 1) * P],
                                            ident_b[:])
                        nc.vector.tensor_copy(mem8[:, d, qb * P:(qb + 1) * P],
                                              pt[:])

            def pos_transposes(pos_T):
                for qi in range(2):
                    for half in range(2):
                        ptm = tmqb.tile([P, D], bf16, tag="tmqbb", name="ptm",
                                        bufs=2)
                        nc.sync.dma_start(ptm[:], pos_own[qi, :,
                                                          half * D:(half + 1) * D])
                        for k in range(DT):
                            j = half * DT + k
                            tidx = 2 * j if j < 8 else 2 * (15 - j) + 1
                            pt = psp.tile([P, P], bf16, tag="pb", name="ptb",
                                          bufs=2)
                            nc.tensor.transpose(pt[:], ptm[:, k * P:(k + 1) * P],
                                                ident_b[:])
                            nc.vector.tensor_copy(
                                pos_T[:, tidx, qi * P:(qi + 1) * P], pt[:])
                nc.scalar.activation(posE[:].rearrange("p a b -> p (a b)"),
                                     pos_T[:].rearrange("p a b -> p (a b)"),
                                     AF.Exp)

            with ExitStack() as pA:
                hA = pA.enter_context(tc.tile_pool(name="hA", bufs=1))
                posp = pA.enter_context(tc.tile_pool(name="posp", bufs=1))
                pos_T = posp.tile([P, 16, 256], bf16)
                h18 = hA.tile([P, DT, 512], f8)
                for qb in range(4):
                    nc.sync.dma_start(x_tm[:, qb, :], x_own[qb, :, :])
                mem_transposes()
                pos_transposes(pos_T)
                rms_to_fm(x_tm, h18)
                kv_shard(0, h18)
                emit_ag(0)

                def q1_sink(dt, ps):
                    nc.vector.tensor_scalar_mul(q1_fm[:, dt, :], ps[:],
                                                0.125 / WS)

                proj_fm_dr(WQ8[0], h18, q1_sink)
                # hoist blk-0's first k/v gathers ahead of block-2 prep in the
                # DMA queue (they still wait on AG0 via data deps)
                kh0 = attn_load_kh(0, khp0, 0)
                vhq0 = [attn_load_vq(0, vhp0, 0, b) for b in range(B)]
                kv_shard(1, mem8)
                emit_ag(1)

            # ---- block 1 attention + o-proj + residual
            if stages >= 2:
                attention(0, q1_fm, o1_fm, bias_self=True,
                          pools=(khp0, vhp0), warm=(kh0, vhq0))
            x1_tm = resid.tile([P, 4, D], f32, tag="resid", name="x1_tm")

            def o1_sink(qb, fh, ps):
                nc.vector.tensor_add(x1_tm[:, qb, fh * 512:(fh + 1) * 512], ps[:],
                                     x_tm[:, qb, fh * 512:(fh + 1) * 512])

            if stages >= 3:
                proj_tm_bf(WOb[0], o1_fm, o1_sink)

        # ================= block 2: cross attention =================
        def block2():
            with ExitStack() as p2:
                q2p = p2.enter_context(tc.tile_pool(name="q2p", bufs=1))
                o2p = p2.enter_context(tc.tile_pool(name="o2p", bufs=1))
                khp1 = p2.enter_context(tc.tile_pool(name="khp1", bufs=2))
                vhp1 = p2.enter_context(tc.tile_pool(name="vhp1", bufs=1))
                q2_fm = q2p.tile([P, DT, 512], bf16)
                o2_fm = o2p.tile([P, DT, 512], bf16)
                # first cross gathers up front: AG1 finished long ago, so
                # these DMAs run during the o1/rms2/q2 transition
                kh1 = attn_load_kh(1, khp1, 0)
                vhq1 = [attn_load_vq(1, vhp1, 0, b) for b in range(B)]

                with ExitStack() as pB:
                    hB = pB.enter_context(tc.tile_pool(name="hB", bufs=1))
                    h28 = hB.tile([P, DT, 512], f8)
                    rms_to_fm(x1_tm, h28, dma_t=True)

                    def q2_sink(dt, ps):
                        nc.vector.tensor_scalar_mul(q2_fm[:, dt, :], ps[:],
                                                    0.125 / WS)

                    proj_fm_dr(WQ8[1], h28, q2_sink)

                if stages >= 5:
                    attention(1, q2_fm, o2_fm, bias_self=False,
                              pools=(khp1, vhp1), warm=(kh1, vhq1))
                x2_tm = resid.tile([P, 4, D], f32, tag="resid", name="x2_tm")

                def o2_sink(qb, fh, ps):
                    nc.vector.tensor_add(x2_tm[:, qb, fh * 512:(fh + 1) * 512],
                                         ps[:],
                                         x1_tm[:, qb, fh * 512:(fh + 1) * 512])

                if stages >= 6:
                    proj_tm_bf(WOb[1], o2_fm, o2_sink)
                return x2_tm

        # ================= block 3: FFN =================
        def block3(x2_tm):
            # FFN fully bf16: fp8 anywhere on this path breaks the error gate
            with ExitStack() as p3:
                zp = p3.enter_context(tc.tile_pool(name="zp", bufs=1))
                w1p = p3.enter_context(tc.tile_pool(name="w1p", bufs=4))
                zb = zp.tile([P, KF, 512], bf16)
                with ExitStack() as pC:
                    hC = pC.enter_context(tc.tile_pool(name="hC", bufs=1))
                    h3b = hC.tile([P, DT, 512], bf16)
                    rms_to_fm(x2_tm, h3b, dma_t=True)
                    w1v = W1b.ap().rearrange("p (k d m) -> p k d m", k=KF, d=DT)
                    for kp in range(16):             # kf pairs, wide psum
                        ps = mmwide()
                        for j in range(2):
                            kf = 2 * kp + j
                            w1t = w1p.tile([P, DT, P], bf16, tag="w1t",
                                           name="w1t")
                            nc.sync.dma_start(w1t[:], w1v[:, kf])
                            for dk in range(DT):
                                nc.tensor.matmul(
                                    ps[:, 512 * j:512 * j + 512],
                                    w1t[:, dk, :], h3b[:, dk, :],
                                    start=(dk == 0), stop=(dk == DT - 1),
                                    skip_group_check=True)
                        nc.scalar.activation(
                            zb[:, 2 * kp:2 * kp + 2, :].rearrange("p a b -> p (a b)"),
                            ps[:], AF.Relu)

                x3tp = p3.enter_context(tc.tile_pool(name="x3tp", bufs=3))
                for fh in range(2):
                    psw = [mmwide() for _ in range(2)]
                    for dk in range(KF):
                        wr = wrhsp.tile([P, 512], bf16, tag="wrhsb", name="w2r",
                                        bufs=9)
                        nc.sync.dma_start(
                            wr[:], W2b[dk * P:(dk + 1) * P,
                                       fh * 512:(fh + 1) * 512])
                        for qb in range(4):
                            nc.tensor.matmul(
                                psw[qb // 2][:, 512 * (qb % 2):
                                             512 * (qb % 2) + 512],
                                zb[:, dk, qb * P:(qb + 1) * P],
                                wr[:], start=(dk == 0), stop=(dk == KF - 1),
                                skip_group_check=True)
                    for qb in range(4):
                        pq = psw[qb // 2][:, 512 * (qb % 2):512 * (qb % 2) + 512]
                        x3t = x3tp.tile([P, 512], f32, tag="x3t", name="x3t")
                        nc.vector.tensor_add(x3t[:], pq,
                                             x2_tm[:, qb, fh * 512:(fh + 1) * 512])
                        nc.sync.dma_start(out[qb, :, fh * 512:(fh + 1) * 512],
                                          x3t[:])

        if stages >= 4:
            x2_tm = block2()
            if stages >= 7:
                block3(x2_tm)

    nc.compile()
    return nc


def _get_nc():
    if "nc" not in _CACHE:
        _CACHE["nc"] = _build()
    return _CACHE["nc"]


def _np_dtypes():
    import concourse.mybir as mybir
    return (mybir.dt.np(mybir.dt.float8e4), mybir.dt.np(mybir.dt.bfloat16))


def _pack_fm(W, T):
    """W [1024, T*128] f32 -> [128, T*4*2*128] fp8 prescaled by WS.
    Layout [p, t, i, j, m] with k = (2i+j)*128+p."""
    E4, _ = _np_dtypes()
    w = (np.asarray(W, np.float32) * WS).reshape(4, 2, P, T, P)
    w = np.ascontiguousarray(w.transpose(2, 3, 0, 1, 4)).reshape(P, -1)
    return w.astype(E4)


def _pack_tm(W, ni):
    """W [ni*2*128, 1024] f32 -> [128, 2*ni*2*512] fp8 prescaled by WS.
    Layout [p, f, i, j, n] with k = (2i+j)*128+p, n = f*512+nn."""
    E4, _ = _np_dtypes()
    w = (np.asarray(W, np.float32) * WS).reshape(ni, 2, P, 2, 512)
    w = np.ascontiguousarray(w.transpose(2, 3, 0, 1, 4)).reshape(P, -1)
    return w.astype(E4)


def _in_maps(x, memory, pos, common):
    ar = np.arange(P)
    # multiplicative 0/1 masks: exp(s + m_additive) == exp(s) * m01
    tri = np.where(ar[:, None] > ar[None, :], np.float32(0.0),
                   np.float32(1.0)).astype(np.float32)
    full = np.zeros((P, P), np.float32)
    keep = np.ones((P, P), np.float32)
    maps = []
    for c in range(NC):
        ch = [c, 15 - c]
        x_own = np.stack([x[b, ch[q2] * P:(ch[q2] + 1) * P, :] for b, q2 in BLOCKS])
        mem_own = np.stack([memory[b, ch[q2] * P:(ch[q2] + 1) * P, :]
                            for b, q2 in BLOCKS])
        pos_own = np.stack([pos[ch[qi] * P:(ch[qi] + 1) * P, :] for qi in range(2)])
        sm = np.zeros((2, NC, P, 256), np.float32)
        for s in range(NC):
            sm[0, s, :, 0:P] = keep if s < c else (tri if s == c else full)
            sm[1, s, :, 0:P] = keep if s > c else (tri if s == c else full)
        m = dict(common)
        E4, BF = _np_dtypes()
        m.update(x_own=np.ascontiguousarray(x_own),
                 mem_own=np.ascontiguousarray(mem_own).astype(BF),
                 pos_own=np.ascontiguousarray(pos_own).astype(BF),
                 smask=sm.astype(BF))
        maps.append(m)
    return maps


def kernel(x, memory, position_embedding, casual_mask,
           g1, Wq1, Wk1, Wv1, Wo1,
           g2, Wq2, Wk2, Wv2, Wo2,
           g3, W1, W2):
    from concourse.bass_utils import run_bass_kernel_spmd

    E4, BF = _np_dtypes()
    x = np.asarray(x, np.float32)
    memory = np.asarray(memory, np.float32)
    pos = np.asarray(position_embedding, np.float32).reshape(L, L)
    gc1 = np.asarray(g1, np.float32)[:, None]
    gc2 = np.asarray(g2, np.float32)[:, None]
    gc3 = np.asarray(g3, np.float32)[:, None]
    common = dict(
        Wq8_1=_pack_fm(gc1 * np.asarray(Wq1, np.float32), DT),
        Wk8_1=_pack_fm(gc1 * np.asarray(Wk1, np.float32), DT),
        Wv8_1=_pack_tm(gc1 * np.asarray(Wv1, np.float32), 4),
        Wo1b=np.asarray(Wo1, np.float32).astype(BF),
        Wq8_2=_pack_fm(gc2 * np.asarray(Wq2, np.float32), DT),
        Wk8_2=_pack_fm(np.asarray(Wk2, np.float32), DT),
        Wv8_2=_pack_tm(np.asarray(Wv2, np.float32), 4),
        Wo2b=np.asarray(Wo2, np.float32).astype(BF),
        W1b=np.ascontiguousarray(
            (gc3 * np.asarray(W1, np.float32)).reshape(DT, P, KF, P)
            .transpose(1, 2, 0, 3).reshape(P, -1)).astype(BF),
        W2b=np.asarray(W2, np.float32).astype(BF),
    )
    nc = _get_nc()
    res = run_bass_kernel_spmd(nc, _in_maps(x, memory, pos, common),
                               core_ids=list(range(NC)))

    outp = np.empty((B, L, D), np.float32)
    for c in range(NC):
        ch = [c, 15 - c]
        o = res.results[c]["out"]
        for i, (b, q2) in enumerate(BLOCKS):
            outp[b, ch[q2] * P:(ch[q2] + 1) * P, :] = o[i]
    return outp
